# revision 48
# baseline (speedup 1.0000x reference)
"""Trainium2 Bass kernel for nn_Cell_67894843015282 (DARTS-style NAS cell).

Strategy:
  - All routing/gating logic (masks, node_sum chain, sigmoid gates) depends only
    on small parameter tensors -> computed on host in f32 (jax-on-CPU to match
    the reference bit-for-bit on the step() comparisons).
  - BN affine + channel gates + op coefficients folded into the pointwise conv
    matrices (per-output-channel scale) and hoisted bias vectors; ops with a
    zero coefficient are skipped entirely.
  - Depthwise (x) pointwise conv pairs fused into per-tap 128x128 matrices,
    quantized to fp8e4m3 (x2^5) and executed as DoubleRow matmuls on the
    tensor engine: each matmul carries TWO taps (two K-tiles, split-half
    pairing keeps the window delta >= ~rowlen) at 0.5 cycles/row -> 4x the
    fp32r tap throughput. The second K-tile's window is addressed via a
    strided AP dimension. The 2^-5 unscale folds into the mid-layer
    activation scale and the final state merge. The fp8 wall crosses the
    jax boundary as uint8 (TRN2's XLA verifier rejects jax-level fp8).
  - Inputs are relu'd on host and shipped in bf16; states are bf16; outputs
    return as bf16 and are cast back to f32 on host.
  - ReLU'd states live in persistent per-state padded fp8 buffers (written
    once by the scalar engine); sep-conv mids relu into fp8 mpads.
  - Pools: max and avg via DVE tensor_tensor passes (bf16 2x mode; avg adds
    a reciprocal-count multiply); skip/pool contributions accumulate into an
    SBUF `extra` buffer with DVE scalar_tensor_tensor chains; `extra` is
    seeded with the hoisted biases by the scalar engine.
  - Step-0 weights prefetch during preprocess; output DMAs are deferred two
    steps so they never block the weight-stream queue head.
  - Data parallel over batch: 1 image per NeuronCore, 8 cores.
"""

import numpy as np

B, C, HH, WW = 8, 128, 32, 32
PIX = HH * WW
C_PREV = 512
STEPS, N_EDGES, N_OPS = 4, 14, 8
N_CORES = 8

WSCALE_POW = 5  # wall weights scaled by 2**WSCALE_POW before fp8 quantization
WSCALE = float(2.0**WSCALE_POW)
UNSCALE = float(2.0**-WSCALE_POW)

# ---------------------------------------------------------------------------
# Host-side gating / fusion (the "plan")
# ---------------------------------------------------------------------------


def _f32(x):
    return np.asarray(x, dtype=np.float32)


def _gate_math(inputs):
    """Replicate the data-independent gating chain of the reference in f32.

    Uses jax on CPU when available so the step() threshold comparisons match
    the reference numerics exactly; falls back to numpy.
    """
    try:
        import jax

        cpu = jax.devices("cpu")[0]

        with jax.default_device(cpu):
            import jax.numpy as jnp

            return _gate_math_impl(jnp, jax.nn.sigmoid, inputs, to_np=np.asarray)
    except Exception:

        def np_sig(x):
            return 1.0 / (1.0 + np.exp(-np.asarray(x, np.float32), dtype=np.float32))

        return _gate_math_impl(np, np_sig, inputs, to_np=np.asarray)


def _gate_math_impl(xp, sig, inputs, to_np):
    f32 = np.float32
    weights2 = xp.asarray(inputs["weights2"], dtype=f32)
    thre = xp.asarray(inputs["thre"], dtype=f32)
    mask_default = xp.asarray(inputs["mask_default"])
    kernel_param = xp.asarray(inputs["kernel_param"], dtype=f32)
    mask_k_default = xp.asarray(inputs["mask_k_default"])
    mask_w_default = xp.asarray(inputs["mask_w_default"])
    kernel_pre = xp.asarray(inputs["kernel_pre"], dtype=f32)
    thre_pre = xp.asarray(inputs["thre_pre"], dtype=f32)

    def step(x):
        return (x > 0).astype(f32)

    mdf = mask_default.astype(f32)

    g0 = sig(kernel_pre[0])
    mk0 = step(g0 - thre_pre[0])
    gv0 = to_np(g0 * mk0).astype(f32)
    g1p = sig(kernel_pre[1])
    mk1 = step(g1p - thre_pre[1])
    gv1 = to_np(g1p * mk1).astype(f32)

    n_states = 2
    offset = 0
    m_all = np.zeros((N_EDGES, N_OPS), np.float32)
    for i in range(STEPS):
        n = n_states
        weight_sum = (weights2[offset : offset + n] * mdf[offset : offset + n]).sum()
        for j in range(n):
            e = offset + j
            ns = weight_sum
            m_list = []
            for k in range(N_OPS):
                w = weights2[e, k]
                md = mdf[e, k]
                m = xp.where(
                    md == 0, f32(0.0), xp.where(w != ns, step(w - thre[e, k, 0]), md)
                )
                cond = (md != 0) & (w != ns) & (m == 0)
                m_list.append(m)
                ns = xp.where(cond, ns - w, ns)
            m_vec = xp.stack(m_list)
            weight_sum = (
                weight_sum - (weights2[e] * mdf[e]).sum() + (weights2[e] * m_vec).sum()
            )
            m_all[e] = to_np(m_vec)
        offset += n
        n_states += 1

    coef = to_np(weights2).astype(f32) * m_all

    gates = to_np(sig(kernel_param)).astype(f32)
    t1 = to_np(thre[:, :, 1]).astype(f32)
    t2 = to_np(thre[:, :, 2]).astype(f32)
    mk = (gates - t1[:, :, None] > 0).astype(f32) * (to_np(mask_k_default) != 0)
    mw = (gates - t2[:, :, None] > 0).astype(f32) * (to_np(mask_w_default) != 0)
    return dict(
        gv0=gv0,
        gv1=gv1,
        coef=coef,
        gates=gates,
        mk=mk.astype(np.float32),
        mw=mw.astype(np.float32),
    )


TAPS3 = [(dy, dx) for dy in (-1, 0, 1) for dx in (-1, 0, 1)]
TAPS5 = [(dy, dx) for dy in (-2, -1, 0, 1, 2) for dx in (-2, -1, 0, 1, 2)]
TAPS3D = [(dy, dx) for dy in (-2, 0, 2) for dx in (-2, 0, 2)]
TAPS5D = [(dy, dx) for dy in (-4, -2, 0, 2, 4) for dx in (-4, -2, 0, 2, 4)]


def build_plan(inputs):
    g = _gate_math(inputs)
    coef = g["coef"]

    scale0 = _f32(inputs["pre0_g"]) * g["gv0"]
    bias0 = _f32(inputs["pre0_b"]) * g["gv0"]
    scale1 = _f32(inputs["pre1_g"]) * g["gv1"]
    bias1 = _f32(inputs["pre1_b"]) * g["gv1"]
    wpre0 = (_f32(inputs["pre0_w"]) * scale0[:, None]).T.copy()  # (512,128)
    wpre1 = (_f32(inputs["pre1_w"]) * scale1[:, None]).T.copy()

    state_of_edge = []
    for i in range(STEPS):
        for j in range(2 + i):
            state_of_edge.append((i, j))

    edges = []
    state_bias = np.zeros((6, C), np.float32)
    for e in range(N_EDGES):
        i, j = state_of_edge[e]
        tgt = 2 + i
        ops = {
            "max": float(coef[e, 1]),
            "avg": float(coef[e, 2]),
            "skip": float(coef[e, 3]),
        }
        for k, nm, taps in ((4, "sep3", TAPS3), (5, "sep5", TAPS5)):
            c = float(coef[e, k])
            if c == 0.0:
                ops[nm] = None
                continue
            gate = g["gates"][e, k]
            mk = g["mk"][e, k]
            mw = g["mw"][e, k]
            s1 = _f32(inputs[f"{nm}_g1"][e]) * gate * mk
            bb1 = _f32(inputs[f"{nm}_b1"][e]) * gate * mk
            s2 = c * _f32(inputs[f"{nm}_g2"][e]) * gate * mw
            bb2 = c * _f32(inputs[f"{nm}_b2"][e]) * gate * mw
            state_bias[tgt] += bb2
            if not s2.any() or not (s1.any() or bb1.any()):
                ops[nm] = None
                continue
            ops[nm] = dict(
                taps=taps,
                layers=[
                    dict(
                        dw=_f32(inputs[f"{nm}_dw1"][e]),
                        pw=_f32(inputs[f"{nm}_pw1"][e]),
                        scale=s1,
                    ),
                    dict(
                        dw=_f32(inputs[f"{nm}_dw2"][e]),
                        pw=_f32(inputs[f"{nm}_pw2"][e]),
                        scale=s2,
                    ),
                ],
                bias1=bb1,
            )
        for k, nm, taps in ((6, "dil3", TAPS3D), (7, "dil5", TAPS5D)):
            c = float(coef[e, k])
            if c == 0.0:
                ops[nm] = None
                continue
            gate = g["gates"][e, k]
            mk = g["mk"][e, k]
            s = c * _f32(inputs[f"{nm}_g"][e]) * gate * mk
            state_bias[tgt] += c * _f32(inputs[f"{nm}_b"][e]) * gate * mk
            if not s.any():
                ops[nm] = None
                continue
            ops[nm] = dict(
                taps=list(taps),
                layers=[
                    dict(
                        dw=_f32(inputs[f"{nm}_dw"][e]),
                        pw=_f32(inputs[f"{nm}_pw"][e]),
                        scale=s,
                    )
                ],
            )
        if ops["dil3"] is not None and ops["dil5"] is not None:
            # merge dil3 into dil5: every dil3 offset is also a dil5 offset and
            # both accumulate into the same psum. Keep TAPS5D raster order and
            # record the dil3 layer for fusion at the shared positions.
            lay5 = ops["dil5"]["layers"][0]
            lay5["merge"] = ops["dil3"]["layers"][0]
            lay5["merge_taps"] = list(TAPS3D)
            ops["dil3"] = None
        edges.append(dict(e=e, step=i, src=j, tgt=tgt, ops=ops))

    cnt1 = np.full(HH, 3.0, np.float32)
    cnt1[0] = cnt1[-1] = 2.0
    cnt = np.float32(1.0) / np.outer(cnt1, cnt1).astype(np.float32)
    rcnt = np.broadcast_to(cnt.reshape(1, PIX), (C, PIX)).copy()

    plan = dict(
        edges=edges,
        wpre0=wpre0,
        wpre1=wpre1,
        bias0=bias0,
        bias1=bias1,
        state_bias=state_bias,
        rcnt=rcnt,
    )
    _fuse_weights(plan)
    return plan


def _fuse_weights(plan):
    """Build the fp8 DoubleRow wall: per layer, taps paired in raster order.

    Each pair occupies (C, 2, 128) in the wall: k-tile 0 = fused matrix of
    tap a, k-tile 1 = tap b (zeros for the odd leftover, whose window delta
    is +1 -- always in-bounds thanks to the padded buffers' extra slack row).
    Weights are scaled by 2**WSCALE_POW before quantization.
    """
    import ml_dtypes

    wall_blocks = []
    wall_off = 0  # in 128-column units

    for ed in plan["edges"]:
        for nm in ("sep3", "sep5", "dil3", "dil5"):
            op = ed["ops"][nm]
            if op is None:
                continue
            taps = op["taps"]
            T = len(taps)
            op["emit"] = []
            for li, lay in enumerate(op["layers"]):
                dwf = lay["dw"].reshape(C, T)
                pws = lay["pw"] * lay["scale"][:, None]  # (O, Cin)
                blk3 = dwf[:, :, None] * pws.T[:, None, :]  # (C, T, O)
                if "merge" in lay:
                    mlay = lay["merge"]
                    mdw = mlay["dw"].reshape(C, len(lay["merge_taps"]))
                    mpws = mlay["pw"] * mlay["scale"][:, None]
                    for mt, tap in enumerate(lay["merge_taps"]):
                        ti = taps.index(tap)
                        blk3[:, ti, :] += mdw[:, mt : mt + 1] * mpws.T
                # split-half pairing: tap i with tap i+ceil(T/2) keeps the
                # window delta >= ~rowlen (tiny deltas hard-fault the PE's
                # DoubleRow ifmap fetch).
                n_pairs = (T + 1) // 2
                pairs = []
                pair_w = np.zeros((C, n_pairs, 2, 128), np.float32)
                for pi in range(n_pairs):
                    a = pi
                    b = pi + n_pairs
                    pair_w[:, pi, 0, :] = blk3[:, a, :]
                    if b < T:
                        pair_w[:, pi, 1, :] = blk3[:, b, :]
                        pairs.append((taps[a], taps[b]))
                    else:
                        pairs.append((taps[a], None))
                ent = dict(
                    pe_off=wall_off,
                    pairs=pairs,
                    n_pairs=n_pairs,
                )
                wall_off += n_pairs * 2
                wall_blocks.append(
                    (pair_w * WSCALE).reshape(C, n_pairs * 256)
                )
                op["emit"].append(ent)

    wall = np.concatenate(wall_blocks, axis=1)
    plan["wall_f8"] = wall.astype(ml_dtypes.float8_e4m3fn)
    plan["n_wall_cols"] = wall.shape[1]


# ---------------------------------------------------------------------------
# Numpy executor (host model of the device plan; for correctness testing)
# ---------------------------------------------------------------------------


def run_plan_numpy(plan, s0, s1):
    wall = np.asarray(plan["wall_f8"], dtype=np.float32)

    def pad_img(x, pad, fill=0.0):
        out = np.full((C, HH + 2 * pad, WW + 2 * pad), fill, np.float32)
        out[:, pad : pad + HH, pad : pad + WW] = x
        return out

    def win(xpad, pad, dy, dx):
        return xpad[:, pad + dy : pad + dy + HH, pad + dx : pad + dx + WW].reshape(
            C, PIX
        )

    def layer_out(ent, xpad, pad):
        acc = np.zeros((C, PIX), np.float32)
        for pi, (ta, tb) in enumerate(ent["pairs"]):
            off = (ent["pe_off"] + 2 * pi) * 128
            wa = wall[:, off : off + 128]
            wb = wall[:, off + 128 : off + 256]
            acc += wa.T @ win(xpad, pad, *ta)
            if tb is not None:
                acc += wb.T @ win(xpad, pad, *tb)
        return acc * UNSCALE

    import ml_dtypes

    def q8(x):
        return x.astype(ml_dtypes.float8_e4m3fn).astype(np.float32)

    states = []
    for s, w, bia in (
        (s0, plan["wpre0"], plan["bias0"]),
        (s1, plan["wpre1"], plan["bias1"]),
    ):
        r = np.maximum(s, 0.0)
        h = w.T @ r + bia[:, None]
        states.append(h.astype(np.float32))

    for i in range(STEPS):
        tgt = 2 + i
        acc = np.zeros((C, PIX), np.float32)
        acc += plan["state_bias"][tgt][:, None]
        for ed in plan["edges"]:
            if ed["step"] != i:
                continue
            x = states[ed["src"]].reshape(C, HH, WW)
            ops = ed["ops"]
            if ops["max"] != 0.0:
                xm = pad_img(x, 1, -np.inf)
                m = np.full((C, HH, WW), -np.inf, np.float32)
                for dy in (-1, 0, 1):
                    for dx in (-1, 0, 1):
                        m = np.maximum(
                            m, xm[:, 1 + dy : 1 + dy + HH, 1 + dx : 1 + dx + WW]
                        )
                acc += ops["max"] * m.reshape(C, PIX)
            if ops["avg"] != 0.0:
                xa = pad_img(x, 1, 0.0)
                ssum = np.zeros((C, HH, WW), np.float32)
                for dy in (-1, 0, 1):
                    for dx in (-1, 0, 1):
                        ssum += xa[:, 1 + dy : 1 + dy + HH, 1 + dx : 1 + dx + WW]
                acc += ops["avg"] * (ssum.reshape(C, PIX) * plan["rcnt"])
            if ops["skip"] != 0.0:
                acc += ops["skip"] * x.reshape(C, PIX)
            rp = pad_img(q8(np.maximum(x, 0.0)), 4)
            for nm in ("sep3", "sep5"):
                op = ops[nm]
                if op is None:
                    continue
                mid = layer_out(op["emit"][0], rp, 4)
                mid = np.maximum(mid + op["bias1"][:, None], 0.0)
                mp = pad_img(q8(mid.reshape(C, HH, WW)), 2)
                acc += layer_out(op["emit"][1], mp, 2)
            for nm in ("dil3", "dil5"):
                op = ops[nm]
                if op is None:
                    continue
                acc += layer_out(op["emit"][0], rp, 4)
        states.append(acc)

    return np.stack(states[2:], axis=0)


# ---------------------------------------------------------------------------
# Bass device program
# ---------------------------------------------------------------------------


def build_device_program(plan):
    from contextlib import ExitStack

    import bass_rust
    import concourse.bacc as bacc
    import concourse.mybir as mybir
    import concourse.tile as tile

    F32 = mybir.dt.float32
    F32R = mybir.dt.float32r
    F8 = mybir.dt.float8e4
    AO = mybir.AluOpType
    AF = mybir.ActivationFunctionType
    DR = mybir.MatmulPerfMode.DoubleRow
    AOm, AOa = AO.mult, AO.add

    n_wall_cols = plan["n_wall_cols"]

    BF16 = mybir.dt.bfloat16

    nc = bacc.Bacc("TRN2", target_bir_lowering=False, debug=False)
    d_s0 = nc.dram_tensor("s0b", [4, 128, PIX], BF16, kind="ExternalInput").ap()
    d_s1 = nc.dram_tensor("s1b", [4, 128, PIX], BF16, kind="ExternalInput").ap()
    # uint8 at the DRAM/jax boundary (jax-level fp8 arrays fail the Neuron XLA
    # verifier on TRN2); bitcast to fp8 on-chip.
    U8 = mybir.dt.uint8
    d_wall = nc.dram_tensor("wall", [128, n_wall_cols], U8, kind="ExternalInput").ap()
    d_wpre = nc.dram_tensor("wpre", [128, 1024], BF16, kind="ExternalInput").ap()
    d_btab = nc.dram_tensor("btab", [128, 64], F32, kind="ExternalInput").ap()
    d_rcnt = nc.dram_tensor("rcnt", [128, PIX], F32, kind="ExternalInput").ap()
    d_out = nc.dram_tensor("out", [4, 128, PIX], BF16, kind="ExternalOutput").ap()

    bias_cols = {}
    next_bias = 6
    for ed in plan["edges"]:
        for nm in ("sep3", "sep5"):
            if ed["ops"][nm] is not None:
                bias_cols[(ed["e"], nm)] = next_bias
                next_bias += 1
    assert next_bias <= 64

    used_max, used_avg = set(), set()
    for ed in plan["edges"]:
        if ed["ops"]["max"] != 0.0:
            used_max.add(ed["src"])
        if ed["ops"]["avg"] != 0.0:
            used_avg.add(ed["src"])

    # padded-buffer geometry: one extra slack row so the odd-pair's +1 window
    # delta stays within the tile.
    RP_H, RP_W = 41, 40  # rpad: 32x32 interior at (4,4)
    MP_H, MP_W = 37, 36  # mpad: 32x32 interior at (2,2)

    def pair_rhs(tile_t, rowlen, pad, h, ta, tb):
        """(128, 2, 16, 32) rhs AP for a DoubleRow tap pair on a padded tile."""
        dy0, dx0 = ta
        if tb is None:
            delta = rowlen  # zero weights in k-tile 1; lands in the slack row
        else:
            delta = (tb[0] - dy0) * rowlen + (tb[1] - dx0)
            assert delta >= rowlen - 8, (ta, tb)
        base = tile_t[
            :,
            pad + dy0 + 16 * h : pad + dy0 + 16 * h + 16,
            pad + dx0 : pad + dx0 + 32,
        ]
        rhs = base.unsqueeze(1).broadcast_to((128, 2, 16, 32)).copy()
        part_stride = base.ap[0][0]
        rhs.ap = bass_rust.VecI64Pair(
            [(part_stride, 128), (delta, 2), (rowlen, 16), (1, 32)]
        )
        return rhs

    with tile.TileContext(nc) as tc, ExitStack() as ctx:
        const = ctx.enter_context(tc.tile_pool(name="const", bufs=1))
        stp = ctx.enter_context(tc.tile_pool(name="stp", bufs=1))
        poolp = ctx.enter_context(tc.tile_pool(name="poolp", bufs=1))
        padp = ctx.enter_context(tc.tile_pool(name="padp", bufs=1))
        extrap = ctx.enter_context(tc.tile_pool(name="extrap", bufs=3))
        psum = ctx.enter_context(tc.tile_pool(name="psum", bufs=2, space="PSUM"))

        # persistent per-state relu pads (fp8)
        rpads = []
        for si in range(6):
            t = padp.tile([128, RP_H, RP_W], F8, tag=f"rpad{si}", name=f"rpad{si}")
            nc.gpsimd.memset(t[:].rearrange("p a b -> p (a b)").bitcast(F32), 0.0)
            rpads.append(t)

        n_mpad = 6
        mpads = []
        for mi in range(n_mpad):
            t = padp.tile([128, MP_H, MP_W], F8, tag=f"mpad{mi}", name=f"mpad{mi}")
            nc.gpsimd.memset(t[:].rearrange("p a b -> p (a b)").bitcast(F32), 0.0)
            mpads.append(t)

        btab = const.tile([128, 64], F32, tag="btab", name="btab")
        nc.gpsimd.dma_start(btab[:], d_btab)
        rcnt = const.tile([128, PIX], F32, tag="rcnt", name="rcnt")
        nc.gpsimd.dma_start(rcnt[:], d_rcnt)

        def bias_ap(col):
            return btab[:, col : col + 1]

        wp = ctx.enter_context(tc.tile_pool(name="wp", bufs=4))

        def dma_weights(ent):
            import os as _os
            wt = wp.tile(
                [128, ent["n_pairs"], 2, 128],
                U8,
                tag=f"w{ent['n_pairs']}",
                name="wt",
                bufs=int(_os.environ.get("KERNEL_WBUFS", "12")),
            )
            nc.sync.dma_start(
                wt[:].rearrange("p a b c -> p (a b c)"),
                d_wall[
                    :, ent["pe_off"] * 128 : (ent["pe_off"] + 2 * ent["n_pairs"]) * 128
                ],
            )
            return wt

        def dma_weights_edge(ops):
            return {
                nm: [dma_weights(ent) for ent in ops[nm]["emit"]]
                for nm in ("sep3", "sep5", "dil3", "dil5")
                if ops[nm] is not None
            }

        wall_tiles = {}

        # ---- preprocess (persistent pool; state1's compute is deferred so PE
        # flows straight from state0's matmuls into step-0 edge e0)
        prep_cm = tc.tile_pool(name="prep", bufs=1)
        prep = prep_cm.__enter__()
        states = []
        wpre = prep.tile([128, 1024], BF16, tag="wpre", name="wpre")
        import os as _os2
        if _os2.environ.get("KERNEL_WPRE_LATE") != "1":
            nc.sync.dma_start(wpre[:], d_wpre)
        stages = []
        import os as _os
        fine = _os.environ.get("KERNEL_STAGE_FINE", "1")
        fine = fine if fine in ("1", "2") else ""
        for si, dsrc in enumerate((d_s0, d_s1)):
            stg = {}
            for half in range(2):  # kk pairs (0,1) and (2,3)
                st = prep.tile([128, 2, PIX], BF16, tag="stage", name="stage", bufs=4)
                if fine == "2":
                    for k2 in range(2):
                        for h2 in range(2):
                            nc.sync.dma_start(
                                st[:, k2, 512 * h2 : 512 * (h2 + 1)],
                                dsrc[2 * half + k2][:, 512 * h2 : 512 * (h2 + 1)],
                            )
                elif fine:
                    for k2 in range(2):
                        nc.sync.dma_start(
                            st[:, k2], dsrc[2 * half + k2].rearrange("p x -> p x")
                        )
                else:
                    nc.sync.dma_start(
                        st[:],
                        dsrc[2 * half : 2 * half + 2].rearrange("k p x -> p k x"),
                    )
                stg[half] = st
            stages.append(stg)
            if si == 0 and _os2.environ.get("KERNEL_WPRE_LATE") == "1":
                nc.sync.dma_start(wpre[:], d_wpre)
        # prefetch step-0 edge weights (queue behind the input DMAs)
        for ed in plan["edges"]:
            if ed["step"] == 0:
                wall_tiles[ed["e"]] = dma_weights_edge(ed["ops"])

        def build_state(si):
            # inputs arrive already relu'd (host-side) in bf16; a cheap DVE/ACT
            # copy still stages each chunk (keeps the scheduler's pre pipeline)
            rel = {}
            stg = stages[si]
            for h in range(2):
                for kk in range(4):
                    st = stg[kk // 2][:, kk % 2]
                    rl = prep.tile([128, 512], BF16, tag="relu", name="relu", bufs=8)
                    if kk % 2 == 0:
                        nc.vector.tensor_scalar(
                            rl[:], st[:, 512 * h : 512 * (h + 1)], 0.0, None, op0=AO.max
                        )
                    else:
                        nc.scalar.activation(
                            rl[:], st[:, 512 * h : 512 * (h + 1)], AF.Relu
                        )
                    rel[(kk, h)] = rl
            ps = psum.tile([128, PIX], F32, tag="acc", name="pre_acc")
            for h in range(2):
                for kk in range(4):
                    nc.tensor.matmul(
                        ps[:, 512 * h : 512 * (h + 1)],
                        wpre[:, 512 * si + 128 * kk : 512 * si + 128 * (kk + 1)],
                        rel[(kk, h)][:],
                        start=(kk == 0),
                        stop=(kk == 3),
                    )
            stt = stp.tile([128, HH, WW], BF16, tag=f"state{si}", name=f"state{si}")
            nc.scalar.activation(
                stt[:],
                ps[:].rearrange("p (a b) -> p a b", a=HH),
                AF.Identity,
                bias=bias_ap(si),
            )
            states.append(stt)
            # fill this state's relu pad (fp8)
            nc.scalar.activation(rpads[si][:, 4:36, 4:36], stt[:], AF.Relu)

        build_state(0)
        build_state(1)
        prep_cm.__exit__(None, None, None)

        scratch = ctx.enter_context(tc.tile_pool(name="scratch", bufs=3))

        maxp_cache = {}
        avgp_cache = {}

        def pool_pass(x, out, tmp, op, eng):
            tt = eng.tensor_tensor
            tt(tmp[:, :, 1:31], x[:, :, 0:30], x[:, :, 1:31], op=op)
            tt(tmp[:, :, 1:31], tmp[:, :, 1:31], x[:, :, 2:32], op=op)
            tt(tmp[:, :, 0:1], x[:, :, 0:1], x[:, :, 1:2], op=op)
            tt(tmp[:, :, 31:32], x[:, :, 30:31], x[:, :, 31:32], op=op)
            tt(out[:, 1:31, :], tmp[:, 0:30, :], tmp[:, 1:31, :], op=op)
            tt(out[:, 1:31, :], out[:, 1:31, :], tmp[:, 2:32, :], op=op)
            tt(out[:, 0:1, :], tmp[:, 0:1, :], tmp[:, 1:2, :], op=op)
            tt(out[:, 31:32, :], tmp[:, 30:31, :], tmp[:, 31:32, :], op=op)

        def get_maxp(s):
            # max pool passes on DVE (Pool engine lacks tensor_tensor max)
            if s not in maxp_cache:
                tmp = scratch.tile([128, HH, WW], BF16, tag="ptmp", name="ptmp", bufs=2)
                out = poolp.tile([128, HH, WW], BF16, tag=f"maxp{s}", name=f"maxp{s}")
                pool_pass(states[s], out, tmp, mybir.AluOpType.max, nc.vector)
                maxp_cache[s] = out
            return maxp_cache[s]

        def get_avgp(s):
            # avg pool passes on the Pool engine (tensor_tensor add/mult work)
            import os

            avg_eng = nc.gpsimd if os.environ.get("KERNEL_AVG_ENG") == "pool" else nc.vector
            if s not in avgp_cache:
                tmp = scratch.tile([128, HH, WW], BF16, tag="ptmp", name="ptmp", bufs=2)
                out = poolp.tile([128, HH, WW], BF16, tag=f"avgp{s}", name=f"avgp{s}")
                pool_pass(states[s], out, tmp, mybir.AluOpType.add, avg_eng)
                avg_eng.tensor_tensor(
                    out[:].rearrange("p a b -> p (a b)"),
                    out[:].rearrange("p a b -> p (a b)"),
                    rcnt[:],
                    op=mybir.AluOpType.mult,
                )
                avgp_cache[s] = out
            return avgp_cache[s]

        mpad_rot = [0]
        pending_out = []

        def flush_out(upto_step):
            while pending_out and pending_out[0][0] <= upto_step:
                _, si_, stt_ = pending_out.pop(0)
                so = stt_[:].rearrange("p a b -> p (a b)")
                for h in range(2):
                    nc.sync.dma_start(
                        d_out[si_][:, 512 * h : 512 * (h + 1)],
                        so[:, 512 * h : 512 * (h + 1)],
                    )

        for i in range(STEPS):
            tgt = 2 + i
            flush_out(i - 2)  # out-DMAs for states merged >= 2 steps ago
            step_edges = [ed for ed in plan["edges"] if ed["step"] == i]
            import os as _os
            if _os.environ.get("KERNEL_EDGE_ORDER", "src") == "big":
                newest = 2 + i - 1

                def _work(ed2):
                    tot = 0
                    for nm2 in ("sep3", "sep5", "dil3", "dil5"):
                        if ed2["ops"][nm2] is not None:
                            tot += sum(e2["n_pairs"] for e2 in ed2["ops"][nm2]["emit"])
                    return tot

                step_edges = sorted(
                    step_edges, key=lambda e2: (e2["src"] == newest, -_work(e2))
                )
            n_acc = 0
            any_extra = False
            for ed in step_edges:
                for nm in ("sep3", "sep5", "dil3", "dil5"):
                    op = ed["ops"][nm]
                    if op is None:
                        continue
                    ent = op["emit"][-1]
                    n_acc += 2 * ent["n_pairs"]
                if (
                    ed["ops"]["max"] != 0.0
                    or ed["ops"]["avg"] != 0.0
                    or ed["ops"]["skip"] != 0.0
                ):
                    any_extra = True

            acc = psum.tile([128, PIX], F32, tag="acc", name="acc") if n_acc else None
            extra = None
            if any_extra:
                extra = extrap.tile([128, PIX], F32, tag="extra", name="extra")
                # init extra with the hoisted per-state bias
                nc.scalar.activation(
                    extra[:].rearrange("p (a b) -> p a b", a=HH),
                    rcnt[:].rearrange("p (a b) -> p a b", a=HH),
                    AF.Identity,
                    bias=bias_ap(2 + (tgt - 2)),
                    scale=0.0,
                )
            acc_idx = [0, 0]

            def acc_mm(h, lhsT, rhs):
                nc.tensor.matmul(
                    acc[:, 512 * h : 512 * (h + 1)],
                    lhsT,
                    rhs,
                    start=(acc_idx[h] == 0),
                    stop=(acc_idx[h] == n_acc - 1),
                    perf_mode=DR,
                )
                acc_idx[h] += 1


            def emit_layer(ent, wt, src_t, rowlen, pad, into_acc, mid_ps=None):
                n_mm = 2 * ent["n_pairs"]
                mm_i = [0]
                for pi, (ta, tb) in enumerate(ent["pairs"]):
                    lhsT = wt[:, pi].bitcast(F8)
                    for h in range(2):
                        rhs = pair_rhs(src_t, rowlen, pad, h, ta, tb)
                        if into_acc:
                            acc_mm(h, lhsT, rhs)
                        else:
                            nc.tensor.matmul(
                                mid_ps[h][:],
                                lhsT,
                                rhs,
                                start=(mm_i[0] < 2),
                                stop=(mm_i[0] >= n_mm - 2),
                                perf_mode=DR,
                            )
                        mm_i[0] += 1

            for ed in step_edges:
                s = ed["src"]
                if s >= len(states):
                    build_state(s)  # deferred state-1 preprocess
                ops = ed["ops"]
                x = states[s]
                xf = x[:].rearrange("p a b -> p (a b)")
                stt_op = nc.vector.scalar_tensor_tensor

                live = [
                    nm for nm in ("sep3", "sep5", "dil3", "dil5") if ops[nm] is not None
                ]
                if live:
                    rp = rpads[s]
                    tiles = wall_tiles.pop(ed["e"], None) or dma_weights_edge(ops)
                    # 1) sep L1s into mid psums (mpads can compute while the
                    #    dils below keep the PE busy)
                    sep_state = {}
                    for nm in ("sep3", "sep5"):
                        if nm not in live:
                            continue
                        op = ops[nm]
                        ent1, ent2 = op["emit"]
                        mid = [
                            psum.tile([128, 512], F32, tag="mid", name="mid", bufs=4)
                            for _ in range(2)
                        ]
                        emit_layer(ent1, tiles[nm][0], rp, RP_W, 4, False, mid)
                        mpad = mpads[mpad_rot[0] % n_mpad]
                        mpad_rot[0] += 1
                        for h in range(2):
                            nc.scalar.activation(
                                mpad[:, 2 + 16 * h : 18 + 16 * h, 2:34],
                                mid[h][:].rearrange("p (a b) -> p a b", a=16),
                                AF.Relu,
                                bias=bias_ap(bias_cols[(ed["e"], nm)]),
                                scale=UNSCALE,
                            )
                        sep_state[nm] = (ent2, mpad)
                    # 2) dil layers straight into acc (covers mpad latency)
                    for nm in ("dil3", "dil5"):
                        if nm in live:
                            emit_layer(
                                ops[nm]["emit"][0], tiles[nm][0], rp, RP_W, 4, True
                            )
                    # 3) sep L2s from their mpads
                    for nm in ("sep3", "sep5"):
                        if nm in sep_state:
                            ent2, mpad = sep_state[nm]
                            emit_layer(ent2, tiles[nm][1], mpad, MP_W, 2, True)

                # pool/skip contributions (not on the PE critical path)
                if ops["max"] != 0.0:
                    mp = get_maxp(s)
                    stt_op(
                        extra[:],
                        mp[:].rearrange("p a b -> p (a b)"),
                        ops["max"],
                        extra[:],
                        op0=AOm,
                        op1=AOa,
                    )
                if ops["avg"] != 0.0:
                    ap_ = get_avgp(s)
                    stt_op(
                        extra[:],
                        ap_[:].rearrange("p a b -> p (a b)"),
                        ops["avg"],
                        extra[:],
                        op0=AOm,
                        op1=AOa,
                    )
                if ops["skip"] != 0.0:
                    stt_op(extra[:], xf, ops["skip"], extra[:], op0=AOm, op1=AOa)

            assert acc_idx[0] == n_acc // 2 and acc_idx[1] == n_acc // 2, (
                acc_idx,
                n_acc,
            )

            stt = stp.tile([128, HH, WW], BF16, tag=f"state{tgt}", name=f"state{tgt}")
            sf = stt[:].rearrange("p a b -> p (a b)")
            # state = UNSCALE * acc + extra   (extra was seeded with state_bias)
            for h in range(2):
                nc.vector.scalar_tensor_tensor(
                    sf[:, 512 * h : 512 * (h + 1)],
                    acc[:, 512 * h : 512 * (h + 1)],
                    UNSCALE,
                    extra[:, 512 * h : 512 * (h + 1)],
                    op0=AOm,
                    op1=AOa,
                )
            states.append(stt)
            if tgt < 6:
                nc.scalar.activation(rpads[tgt][:, 4:36, 4:36], stt[:], AF.Relu)

            pending_out.append((i, i, stt))

        flush_out(STEPS)

    nc.compile()
    return nc


def _make_btab(plan):
    btab = np.zeros((128, 64), np.float32)
    btab[:, 0] = plan["bias0"]
    btab[:, 1] = plan["bias1"]
    for i in range(4):
        btab[:, 2 + i] = plan["state_bias"][2 + i]
    col = 6
    for ed in plan["edges"]:
        for nm in ("sep3", "sep5"):
            if ed["ops"][nm] is not None:
                btab[:, col] = ed["ops"][nm]["bias1"]
                col += 1
    return btab


def make_in_maps(plan, inputs):
    import ml_dtypes

    wpre = np.zeros((128, 1024), np.float32)
    wpre[:, 0:512] = (
        plan["wpre0"].reshape(4, 128, 128).transpose(1, 0, 2).reshape(128, 512)
    )
    wpre[:, 512:1024] = (
        plan["wpre1"].reshape(4, 128, 128).transpose(1, 0, 2).reshape(128, 512)
    )
    btab = _make_btab(plan)
    bf16 = ml_dtypes.bfloat16
    s0 = np.maximum(_f32(inputs["s0"]), 0.0).reshape(B, 4, 128, PIX).astype(bf16)
    s1 = np.maximum(_f32(inputs["s1"]), 0.0).reshape(B, 4, 128, PIX).astype(bf16)
    base = {
        "wall": np.ascontiguousarray(plan["wall_f8"]).view(np.uint8),
        "wpre": wpre.astype(bf16),
        "btab": btab,
        "rcnt": plan["rcnt"],
    }
    return [
        {
            **base,
            "s0b": np.ascontiguousarray(s0[b]),
            "s1b": np.ascontiguousarray(s1[b]),
        }
        for b in range(B)
    ]


def kernel(**inputs):
    import os

    plan = build_plan(inputs)

    if os.environ.get("KERNEL_NUMPY") == "1":
        s0 = _f32(inputs["s0"]).reshape(B, C_PREV, PIX)
        s1 = _f32(inputs["s1"]).reshape(B, C_PREV, PIX)
        outs = []
        for b in range(B):
            r = run_plan_numpy(plan, s0[b], s1[b])
            outs.append(r.reshape(4 * C, HH, WW))
        return np.stack(outs).astype(np.float32)

    from concourse.bass_utils import run_bass_kernel_spmd

    nc = build_device_program(plan)
    in_maps = make_in_maps(plan, inputs)
    res = run_bass_kernel_spmd(nc, in_maps, core_ids=list(range(N_CORES)))
    out = np.stack(
        [
            np.asarray(res.results[b]["out"]).astype(np.float32).reshape(4 * C, HH, WW)
            for b in range(B)
        ]
    )
    return out.astype(np.float32)
